# revision 3
# baseline (speedup 1.0000x reference)
"""DGCNN forward kernel for Trainium2, 8 NeuronCores, pure data parallelism.

Math (per graph b):  h_b = relu(sum_k T_k @ X_b @ W_k);  out_b = h_b.flat @ fc_w + fc_b
with T_k the (symmetric) Chebyshev supports of the normalized adjacency.

v2 design (bf16 data path, fp32 PSUM accumulation):
  - x is host-permuted to [chunk, (j,m)=128, grp, pair, c] bf16 so each chunk is
    one 2 MiB fully-contiguous DMA (16 KiB per partition line).
  - stage A (T-contraction) is row-tiled: per batch-pair, two concurrent
    64-row matmuls (graph j in array rows 64j..64j+63), each streaming
    T_all [64, (k,n)=256] -> Q^T[c, (k,n)] per graph. PE cost halves vs the
    128-row blockdiag form.
  - Q copies PSUM->SBUF bf16, alternating DVE/ACT.
  - stage B: per 4-pair group, 4 PSUM-accumulating matmuls over k,
    lhsT = W_k [c, o], rhs = Q[:, k] contiguous [128, 512].
  - relu PSUM->SBUF bf16 into h_t [o, batch, n] (batch-major), alternating engines.
  - stage C at the end: 64 matmuls (one per n), FD=512 batches,
    PSUM-accumulate over n -> logits [3, 512]; bias add; one small DMA out.
"""

import sys

if "/opt/trn_rl_repo" not in sys.path:
    sys.path.insert(0, "/opt/trn_rl_repo")

from contextlib import ExitStack

import ml_dtypes
import numpy as np

import concourse.bass as bass
import concourse.tile as tile
from concourse import bacc
from concourse import mybir
from concourse.bass_utils import run_bass_kernel_spmd

B, N, C, K, O, CLS = 4096, 64, 128, 4, 64, 3
NCORES = 8
BS = B // NCORES  # 512 graphs per core
NCHUNKS = 4  # x DMA chunks per core
GP = 4  # pairs per group (stage B free dim = GP*2*N = 512)
CG = BS // (NCHUNKS * GP * 2)  # groups per chunk = 16

f32 = mybir.dt.float32
bf16 = mybir.dt.bfloat16


def _host_constants(A, gc_w, fc_w):
    """Mirror reference.py's normalize_A + Chebyshev supports in fp32 numpy."""
    eye = np.eye(N, dtype=np.float32)
    Ar = np.maximum(A, 0) * (1 - eye)
    Ar = Ar + Ar.T
    d = (1.0 / np.sqrt(Ar.sum(axis=1) + 1e-10)).astype(np.float32)
    L = eye - (d[:, None] * Ar) * d[None, :]
    Ln = (L - eye).astype(np.float32)  # 2L/2 - I
    sup = [eye, Ln]
    for _ in range(2, K):
        sup.append(2.0 * Ln @ sup[-1] - sup[-2])
    T = np.stack(sup[:K]).astype(np.float32)  # [K,N,N], each symmetric

    T_all = np.zeros((N, K * N), np.float32)  # [m,(k,n)]
    for k in range(K):
        T_all[:, k * N : (k + 1) * N] = T[k]
    BDT2 = np.concatenate([T_all, T_all], axis=0)  # [(j,m)=128, (k,n)=256]
    W_km = np.ascontiguousarray(gc_w.transpose(1, 0, 2)).reshape(C, K * O)  # [c,(k,o)]
    F_sb = np.ascontiguousarray(fc_w.reshape(N, O, CLS).transpose(1, 0, 2)).reshape(
        O, N * CLS
    )  # [o,(n,cls)]
    return (
        BDT2.astype(ml_dtypes.bfloat16),
        W_km.astype(ml_dtypes.bfloat16),
        F_sb.astype(ml_dtypes.bfloat16),
    )


def _pack_x(xc):
    """[BS, N, C] f32 (one core's graphs) -> [NCHUNKS, 2N, CG*GP*C] bf16 with
    layout [chunk, (j,m), grp, pair, c] so each chunk DMA is fully contiguous."""
    a = xc.reshape(NCHUNKS, CG, GP, 2, N, C)
    a = a.transpose(0, 3, 4, 1, 2, 5)  # [chunk, j, m, grp, pair, c]
    return np.ascontiguousarray(a).reshape(NCHUNKS, 2 * N, CG * GP * C).astype(
        ml_dtypes.bfloat16
    )


def prepare_core_inputs(x, A, gc_w, fc_w, fc_b):
    """Full inputs -> dict of per-core stacked arrays [NCORES, ...]."""
    x = np.ascontiguousarray(x, dtype=np.float32)
    BDT2, W_km, F_sb = _host_constants(
        np.asarray(A, np.float32),
        np.asarray(gc_w, np.float32),
        np.asarray(fc_w, np.float32),
    )
    bias = np.asarray(fc_b, np.float32).reshape(CLS, 1)
    shards = x.reshape(NCORES, BS, N, C)
    xp = np.stack([_pack_x(shards[i]) for i in range(NCORES)])
    return {
        "x": xp,
        "bdt": np.broadcast_to(BDT2, (NCORES, *BDT2.shape)),
        "w": np.broadcast_to(W_km, (NCORES, *W_km.shape)),
        "f": np.broadcast_to(F_sb, (NCORES, *F_sb.shape)),
        "b": np.broadcast_to(bias, (NCORES, *bias.shape)),
    }


def build(bs, loop_reps=0, reps=1, dma_only=False, xp_bufs=2, qp_bufs=3, hp_bufs=2,
          psa_bufs=2, psb_bufs=2):
    """Build the SPMD Bass program for a per-core shard of `bs` graphs."""
    assert bs == BS
    nc = bacc.Bacc()
    x_in = nc.declare_dram_parameter("x", [NCHUNKS, 2 * N, CG * GP * C], bf16, isOutput=False)
    bdt_in = nc.declare_dram_parameter("bdt", [2 * N, K * N], bf16, isOutput=False)
    w_in = nc.declare_dram_parameter("w", [C, K * O], bf16, isOutput=False)
    f_in = nc.declare_dram_parameter("f", [O, N * CLS], bf16, isOutput=False)
    b_in = nc.declare_dram_parameter("b", [CLS, 1], f32, isOutput=False)
    out_ext = nc.declare_dram_parameter("out", [CLS, bs], f32, isOutput=True)

    with ExitStack() as ctx:
        tc = ctx.enter_context(tile.TileContext(nc))
        consts = ctx.enter_context(tc.tile_pool(name="consts", bufs=1))
        xp = ctx.enter_context(tc.tile_pool(name="xp", bufs=xp_bufs))
        qp = ctx.enter_context(tc.tile_pool(name="qp", bufs=qp_bufs))
        hp = ctx.enter_context(tc.tile_pool(name="hp", bufs=hp_bufs))
        outp = ctx.enter_context(tc.tile_pool(name="outp", bufs=2))
        psA = ctx.enter_context(tc.tile_pool(name="psA", bufs=psa_bufs, space="PSUM"))
        psB = ctx.enter_context(tc.tile_pool(name="psB", bufs=psb_bufs, space="PSUM"))
        psC = ctx.enter_context(tc.tile_pool(name="psC", bufs=1, space="PSUM"))

        bdt_t = consts.tile([2 * N, K, N], bf16)
        nc.sync.dma_start(out=bdt_t, in_=bdt_in[:].rearrange("p (k n) -> p k n", n=N))
        w_t = consts.tile([C, K, O], bf16)
        nc.sync.dma_start(out=w_t, in_=w_in[:].rearrange("c (k o) -> c k o", o=O))
        f_t = consts.tile([O, N, CLS], bf16)
        nc.sync.dma_start(out=f_t, in_=f_in[:].rearrange("o (n cls) -> o n cls", cls=CLS))
        bias_t = consts.tile([CLS, 1], f32)
        nc.sync.dma_start(out=bias_t, in_=b_in[:])

        rep_ctx = (
            tc.For_i(0, loop_reps, 1, hint_engines=tuple(nc.engines))
            if loop_reps
            else None
        )
        if rep_ctx is not None:
            ctx.enter_context(rep_ctx)
        for _rep in range(reps):
            h_t = hp.tile([O, bs, N], bf16)
            for ch in range(NCHUNKS):
                x_t = xp.tile([2 * N, CG, GP, C], bf16)
                nc.sync.dma_start(
                    out=x_t,
                    in_=x_in[ch].rearrange("p (g pr c) -> p g pr c", g=CG, pr=GP),
                )
                if dma_only:
                    continue
                for g in range(CG):
                    q_t = qp.tile([C, K, GP, 2, N], bf16)
                    for pr in range(GP):
                        qa0 = psA.tile([C, K, N], f32)
                        qa1 = psA.tile([C, K, N], f32)
                        nc.tensor.matmul(
                            out=qa0, lhsT=x_t[0:N, g, pr, :], rhs=bdt_t[0:N],
                            start=True, stop=True,
                        )
                        nc.tensor.matmul(
                            out=qa1, lhsT=x_t[N : 2 * N, g, pr, :], rhs=bdt_t[N : 2 * N],
                            start=True, stop=True,
                        )
                        nc.vector.tensor_copy(out=q_t[:, :, pr, 0, :], in_=qa0)
                        nc.scalar.copy(out=q_t[:, :, pr, 1, :], in_=qa1)
                    hb = psB.tile([O, GP, 2, N], f32)
                    for k in range(K):
                        nc.tensor.matmul(
                            out=hb, lhsT=w_t[:, k, :], rhs=q_t[:, k],
                            start=(k == 0), stop=(k == K - 1),
                        )
                    gb = (ch * CG + g) * (GP * 2)
                    if g % 2 == 0:
                        nc.vector.tensor_relu(out=h_t[:, gb : gb + GP * 2, :], in_=hb)
                    else:
                        nc.scalar.activation(
                            out=h_t[:, gb : gb + GP * 2, :], in_=hb,
                            func=mybir.ActivationFunctionType.Relu,
                        )
            if dma_only:
                continue
            oc = psC.tile([CLS, bs], f32)
            for n in range(N):
                nc.tensor.matmul(
                    out=oc, lhsT=f_t[:, n, :], rhs=h_t[:, :, n],
                    start=(n == 0), stop=(n == N - 1),
                )
            ost = outp.tile([CLS, bs], f32)
            nc.vector.tensor_scalar_add(out=ost, in0=oc, scalar1=bias_t[:])
            nc.sync.dma_start(out=out_ext[:], in_=ost)

    nc.compile()
    return nc


def run(x, A, gc_w, fc_w, fc_b, trace=False):
    per_core = prepare_core_inputs(x, A, gc_w, fc_w, fc_b)
    nc = build(BS)
    in_maps = [{k: v[i] for k, v in per_core.items()} for i in range(NCORES)]
    br = run_bass_kernel_spmd(nc, in_maps, list(range(NCORES)), trace=trace)
    outs = [br.results[i]["out"].T for i in range(NCORES)]  # each [bs, CLS]
    return np.concatenate(outs, axis=0).astype(np.float32), br


def kernel(x, A, gc_w, fc_w, fc_b):
    out, _ = run(x, A, gc_w, fc_w, fc_b)
    return out


if __name__ == "__main__":
    rng = np.random.default_rng(0)
    x = rng.standard_normal((B, N, C), dtype=np.float32)
    A = rng.uniform(0.01, 0.5, (N, N)).astype(np.float32)
    gc_w = (rng.standard_normal((K, C, O), dtype=np.float32) * 0.1).astype(np.float32)
    fc_w = (rng.standard_normal((N * O, CLS), dtype=np.float32) * 0.02).astype(np.float32)
    fc_b = np.zeros(CLS, np.float32)
    out = kernel(x=x, A=A, gc_w=gc_w, fc_w=fc_w, fc_b=fc_b)
    print(out.shape, out.dtype)
